# revision 2
# baseline (speedup 1.0000x reference)
"""Causal self-attention (B=2, S=4096, D=512, H=8) on 8 Trainium2 NeuronCores.

Sharding: tensor-parallel over heads. Core h computes head h for both batch
elements: QKV projections for its head, causal flash attention, and its
partial (unnormalized) o_proj contribution y_h = U_h @ Wo[h*64:(h+1)*64, :]
plus the per-query softmax denominators L_h. The host computes
sum_h(y_h / L_h) + bo.

All matmuls run in bf16 (fp32 PSUM accumulation). Score matmuls have K=64
(head dim), so consecutive k-tiles are packed onto the two 64-row groups of
the PE array via tile_position (auto-derived from base partitions) and run
concurrently:
  - qt2 [128, 4096]/batch: Q.T duplicated in partition halves 0:64 / 64:128.
  - ktp [128, 2048]/batch: even k-tiles' K.T in partitions 0:64, odd in
    64:128; column block j*128:(j+1)*128 holds the pair (2j, 2j+1).
  - chunk j: two concurrent matmuls -> st psum [128, 2, 512] (two banks),
    one ACT exp [128, 1024] PSUM->SBUF -> P.T bf16; diagonal chunks get a
    0/1 causal mask multiply on DVE.
  - AV: U'[65, 512] += V'_kt.T @ P.T_kt with V' = [V | ones]; row 64
    accumulates L. AV for chunk j is emitted after the score pair of chunk
    j+1 so each pair stays adjacent in the PE stream.
  - o_proj: y tiles [128q, 512] = U'.T chunk @ Wo_h with K=65 (Wo row 64
    zeroed on host so the L row contributes nothing), bf16 out, DMA'd
    unnormalized together with L (bf16).
"""

import sys

for _p in ("/opt/trn_rl_repo", "/root/.axon_site/_ro/trn_rl_repo"):
    if _p not in sys.path:
        sys.path.insert(0, _p)

import numpy as np

import concourse.bass as bass
import concourse.mybir as mybir
import concourse.tile as tile
from concourse import bacc
from concourse.bass_utils import run_bass_kernel_spmd

B = 2
S = 4096
D = 512
H = 8
HD = 64
TOK = B * S          # 8192
NKT = S // 128       # 32 k-tiles per batch
SCALE = HD ** -0.5

F32 = mybir.dt.float32
BF16 = mybir.dt.bfloat16

_CACHE = {}


def _build():
    nc = bacc.Bacc("TRN2", target_bir_lowering=False, debug=False, num_devices=8)

    xt_d = nc.dram_tensor("xt", [D, TOK], BF16, kind="ExternalInput")
    wqk_d = nc.dram_tensor("wqk", [D, 128], BF16, kind="ExternalInput")
    wv_d = nc.dram_tensor("wv", [D, HD], BF16, kind="ExternalInput")
    wo_d = nc.dram_tensor("wo", [65, D], BF16, kind="ExternalInput")
    bqk_d = nc.dram_tensor("bqk", [128, 1], F32, kind="ExternalInput")
    bv_d = nc.dram_tensor("bv", [HD, 1], F32, kind="ExternalInput")
    mask_d = nc.dram_tensor("mask", [128, 4, 512], BF16, kind="ExternalInput")
    identb_d = nc.dram_tensor("identb", [64, 64], BF16, kind="ExternalInput")
    onesb_d = nc.dram_tensor("onesb", [128, NKT], BF16, kind="ExternalInput")
    y_d = nc.dram_tensor("y", [TOK, D], BF16, kind="ExternalOutput")
    l_d = nc.dram_tensor("l", [TOK], BF16, kind="ExternalOutput")

    xt_r = xt_d.ap().rearrange("(c p) t -> p c t", p=128)      # [128, 4, 8192]
    wqk_r = wqk_d.ap().rearrange("(c p) m -> p c m", p=128)    # [128, 4, 128]
    wv_r = wv_d.ap().rearrange("(c p) m -> p c m", p=128)      # [128, 4, 64]

    with tile.TileContext(nc) as tc:
        import contextlib

        with contextlib.ExitStack() as ctx:
            singles = ctx.enter_context(tc.tile_pool(name="singles", bufs=1))
            xpool = ctx.enter_context(tc.tile_pool(name="xt", bufs=3))
            ptpool = ctx.enter_context(tc.tile_pool(name="pt", bufs=4))
            upool = ctx.enter_context(tc.tile_pool(name="usb", bufs=2))
            ypool = ctx.enter_context(tc.tile_pool(name="ysb", bufs=4))
            kstpool = ctx.enter_context(tc.tile_pool(name="kst", bufs=2))

            ps_st = ctx.enter_context(
                tc.tile_pool(name="ps_st", bufs=2, space="PSUM")
            )
            ps_u = ctx.enter_context(tc.tile_pool(name="ps_u", bufs=2, space="PSUM"))
            ps_misc = ctx.enter_context(
                tc.tile_pool(name="ps_misc", bufs=2, space="PSUM")
            )

            # --- constants / weights -----------------------------------
            wqk_sb = singles.tile([128, 4, 128], BF16)
            wv_sb = singles.tile([128, 4, HD], BF16)
            wo_sb = singles.tile([65, D], BF16)
            bqk_sb = singles.tile([128, 1], F32)
            bv_sb = singles.tile([HD, 1], F32)
            mask_sb = singles.tile([128, 4, 512], BF16)
            identb = singles.tile([64, 64], BF16)
            nc.sync.dma_start(out=wqk_sb, in_=wqk_r)
            nc.sync.dma_start(out=wv_sb, in_=wv_r)
            nc.sync.dma_start(out=wo_sb, in_=wo_d.ap())
            nc.sync.dma_start(out=bqk_sb, in_=bqk_d.ap())
            nc.sync.dma_start(out=bv_sb, in_=bv_d.ap())
            nc.sync.dma_start(out=mask_sb, in_=mask_d.ap())
            nc.sync.dma_start(out=identb, in_=identb_d.ap())

            # --- persistent per-batch activation buffers ---------------
            qt2 = [
                singles.tile([128, S], BF16, tag=f"qt2_{b}", name=f"qt2_{b}")
                for b in range(B)
            ]
            ktp = [
                singles.tile([128, S // 2], BF16, tag=f"ktp_{b}", name=f"ktp_{b}")
                for b in range(B)
            ]
            vp = [
                singles.tile([128, NKT * 65], BF16, tag=f"vp_{b}", name=f"vp_{b}")
                for b in range(B)
            ]
            for b in range(B):
                nc.sync.dma_start(
                    out=vp[b].rearrange("p (t c) -> p t c", c=65)[:, :, 64:65],
                    in_=onesb_d.ap().rearrange("p (t c) -> p t c", c=1),
                )

            def proj_block(b, tb):
                """Projections for 512 tokens (block tb of batch b)."""
                t0 = b * S + tb * 512
                xt_sb = xpool.tile([128, 4, 512], BF16, tag="xt")
                nc.sync.dma_start(out=xt_sb, in_=xt_r[:, :, t0 : t0 + 512])

                qk_ps = ps_misc.tile([128, 512], F32, tag="m")
                for c in range(4):
                    nc.tensor.matmul(
                        qk_ps,
                        wqk_sb[:, c, :],
                        xt_sb[:, c, :],
                        start=(c == 0),
                        stop=(c == 3),
                    )
                vt_ps = ps_misc.tile([HD, 512], F32, tag="m")
                for c in range(4):
                    nc.tensor.matmul(
                        vt_ps,
                        wv_sb[:, c, :],
                        xt_sb[:, c, :],
                        start=(c == 0),
                        stop=(c == 3),
                    )

                # Q.T -> QT2 top half (+bias), then DMA-dup to bottom half
                cols = slice(tb * 512, (tb + 1) * 512)
                nc.vector.tensor_scalar_add(
                    qt2[b][0:64, cols], qk_ps[0:64, :], bqk_sb[0:64, 0:1]
                )
                nc.sync.dma_start(out=qt2[b][64:128, cols], in_=qt2[b][0:64, cols])

                # K.T (+bias) -> ktmp rows 64:128, then DMA-scatter into the
                # even/odd packed layout of ktp (pair (2j, 2j+1) shares cols)
                ktmp = kstpool.tile([128, 512], BF16, tag="ktmp")
                nc.vector.tensor_scalar_add(
                    ktmp[64:128, :], qk_ps[64:128, :], bqk_sb[64:128, 0:1]
                )
                ksrc = ktmp[64:128, :].rearrange("p (a b c) -> p a b c", b=2, c=128)
                kdst = ktp[b][:, tb * 256 : (tb + 1) * 256].rearrange(
                    "p (a c) -> p a c", c=128
                )
                nc.sync.dma_start(out=kdst[0:64], in_=ksrc[:, :, 0, :])
                nc.sync.dma_start(out=kdst[64:128], in_=ksrc[:, :, 1, :])

                # V.T (+bias, bf16) -> PE transpose to V natural -> V' blocks
                vt_sb = kstpool.tile([HD, 512], BF16, tag="vt")
                nc.vector.tensor_scalar_add(vt_sb, vt_ps, bv_sb[:, 0:1])
                for j in range(4):
                    kt = tb * 4 + j
                    vtr_ps = ps_misc.tile([128, HD], BF16, tag="m")
                    nc.tensor.transpose(
                        vtr_ps, vt_sb[:, j * 128 : (j + 1) * 128], identb
                    )
                    nc.vector.tensor_copy(vp[b][:, kt * 65 : kt * 65 + 64], vtr_ps)

            def attn_qblock(b, qb):
                """Attention + unnormalized o_proj for q-block qb of batch b."""
                q0 = qb * 512
                u_ps = ps_u.tile([65, 512], F32, tag="u")
                n_chunks = 2 * (qb + 1)  # chunks of 2 k-tiles

                def emit_av(pt, j):
                    for j2 in range(2):
                        kt = 2 * j + j2
                        nc.tensor.matmul(
                            u_ps,
                            vp[b][:, kt * 65 : kt * 65 + 65],
                            pt[:, j2, :],
                            start=(kt == 0),
                            stop=(kt == 2 * n_chunks - 1),
                            skip_group_check=True,
                        )

                prev_pt = None
                for j in range(n_chunks):
                    st = ps_st.tile([128, 2, 512], F32, tag="st")
                    kcols = slice(j * 128, (j + 1) * 128)
                    qcols = slice(q0, q0 + 512)
                    nc.tensor.matmul(
                        st[:, 0, :],
                        ktp[b][0:64, kcols],
                        qt2[b][0:64, qcols],
                        start=True,
                        stop=True,
                    )
                    nc.tensor.matmul(
                        st[:, 1, :],
                        ktp[b][64:128, kcols],
                        qt2[b][64:128, qcols],
                        start=True,
                        stop=True,
                    )
                    pt = ptpool.tile([128, 2, 512], BF16, tag="pt")
                    nc.scalar.activation(
                        pt, st, mybir.ActivationFunctionType.Exp, scale=SCALE
                    )
                    if j >= n_chunks - 2:  # diagonal chunks: causal mask
                        d0 = (j % 2) * 2
                        nc.vector.tensor_mul(pt, pt, mask_sb[:, d0 : d0 + 2, :])
                    if prev_pt is not None:
                        emit_av(prev_pt, j - 1)
                    prev_pt = pt
                emit_av(prev_pt, n_chunks - 1)

                # U' -> SBUF [65, 512] bf16 (U rows 0:64, L row 64)
                u_sb = upool.tile([65, 512], BF16, tag="u")
                nc.vector.tensor_copy(u_sb, u_ps)

                row0 = b * S + q0
                nc.sync.dma_start(
                    out=l_d.ap()[row0 : row0 + 512].rearrange("(p c) -> p c", p=1),
                    in_=u_sb[64:65, :],
                )

                # y = U'.T @ Wo_h (unnormalized); K=65 with wo row 64 = 0
                for j2 in range(4):
                    y_ps = ps_misc.tile([128, 512], F32, tag="m")
                    nc.tensor.matmul(
                        y_ps,
                        u_sb[:, j2 * 128 : (j2 + 1) * 128],
                        wo_sb,
                        start=True,
                        stop=True,
                    )
                    y_sb = ypool.tile([128, 512], BF16, tag="y")
                    nc.vector.tensor_copy(y_sb, y_ps)
                    r0 = row0 + j2 * 128
                    nc.sync.dma_start(out=y_d.ap()[r0 : r0 + 128, :], in_=y_sb)

            # Pipeline: proj(tb) immediately enables attn(qb=tb).
            for b in range(B):
                for tb in range(8):
                    proj_block(b, tb)
                    attn_qblock(b, tb)

    nc.compile()
    return nc


def _prep_inputs(x, Wq, bq, Wk, bk, Wv, bv, Wo, bo):
    import ml_dtypes

    bf = ml_dtypes.bfloat16
    xt = np.ascontiguousarray(x.reshape(TOK, D).T).astype(bf)
    mask = np.zeros((128, 4, 512), dtype=np.float32)
    p = np.arange(128)[:, None]
    c = np.arange(512)[None, :]
    for d in range(4):
        mask[:, d, :] = (p + 128 * d <= c).astype(np.float32)
    mask = mask.astype(bf)
    identb = np.eye(64, dtype=np.float32).astype(bf)
    onesb = np.ones((128, NKT), dtype=np.float32).astype(bf)

    in_maps = []
    for h in range(H):
        hs = slice(h * HD, (h + 1) * HD)
        wo_h = np.concatenate(
            [Wo[hs, :], np.zeros((1, D), dtype=np.float32)], axis=0
        )
        in_maps.append(
            {
                "xt": xt,
                "wqk": np.ascontiguousarray(
                    np.concatenate([Wq[:, hs], Wk[:, hs]], axis=1)
                ).astype(bf),
                "wv": np.ascontiguousarray(Wv[:, hs]).astype(bf),
                "wo": wo_h.astype(bf),
                "bqk": np.concatenate([bq[hs], bk[hs]]).reshape(128, 1).astype(
                    np.float32
                ),
                "bv": bv[hs].reshape(HD, 1).astype(np.float32),
                "mask": mask,
                "identb": identb,
                "onesb": onesb,
            }
        )
    return in_maps


def _install_ntff_hook():
    """Register the axon NTFF profiling hook (test-only plumbing)."""
    import types

    try:
        from antenv.axon_hooks import set_axon_ntff_profile_hook  # noqa: F401
    except ImportError:
        m = types.ModuleType("antenv.axon_hooks")
        m._HOOK = None
        m.set_axon_ntff_profile_hook = lambda h: setattr(m, "_HOOK", h)
        m.get_axon_ntff_profile_hook = lambda: m._HOOK
        sys.modules["antenv.axon_hooks"] = m
        import antenv

        antenv.axon_hooks = m
    from antenv.axon_hooks import (
        get_axon_ntff_profile_hook,
        set_axon_ntff_profile_hook,
    )

    if get_axon_ntff_profile_hook() is None:
        import trn_agent_boot.trn_boot as tb

        set_axon_ntff_profile_hook(
            tb._ntff_profile_via_ctypes("/opt/axon/libaxon_pjrt.so")
        )


def kernel(x, Wq, bq, Wk, bk, Wv, bv, Wo, bo, _trace=False):
    x, Wq, bq, Wk, bk, Wv, bv, Wo, bo = (
        np.asarray(a, dtype=np.float32) for a in (x, Wq, bq, Wk, bk, Wv, bv, Wo, bo)
    )
    if "nc" not in _CACHE:
        _CACHE["nc"] = _build()
    nc = _CACHE["nc"]
    in_maps = _prep_inputs(x, Wq, bq, Wk, bk, Wv, bv, Wo, bo)
    kwargs = {}
    if _trace:
        _install_ntff_hook()
        kwargs = dict(trace=True, trace_cores=[0])
    res = run_bass_kernel_spmd(nc, in_maps, core_ids=list(range(8)), **kwargs)
    _CACHE["last_result"] = res
    y = np.zeros((TOK, D), dtype=np.float64)
    for r in res.results:
        y += r["y"].astype(np.float64) / r["l"].astype(np.float64)[:, None]
    y += bo[None, :]
    return y.astype(np.float32).reshape(B, S, D)


# revision 20
# speedup vs baseline: 1.0113x; 1.0113x over previous
"""Causal self-attention (B=2, S=4096, D=512, H=8) on 8 Trainium2 NeuronCores.

Sharding: tensor-parallel over heads. Core h computes head h for both batch
elements: QKV projections for its head, causal flash attention, and its
partial (unnormalized) o_proj contribution y_h = U_h @ Wo[h*64:(h+1)*64, :]
plus the per-query softmax denominators L_h. The host computes
sum_h(y_h / L_h) + bo.

All matmuls run in bf16 (fp32 PSUM accumulation). Score matmuls have K=64
(head dim), so consecutive k-tiles are packed onto the two 64-row groups of
the PE array via tile_position (auto-derived from base partitions) and run
concurrently:
  - qt2 [128, 4096]/batch: Q.T duplicated in partition halves 0:64 / 64:128.
  - ktp [128, 2048]/batch: even k-tiles' K.T in partitions 0:64, odd in
    64:128; column block j*128:(j+1)*128 holds the pair (2j, 2j+1).
  - chunk j: two concurrent matmuls -> st psum [128, 2, 512] (two banks),
    one ACT exp [128, 1024] PSUM->SBUF -> P.T bf16; diagonal chunks get a
    0/1 causal mask multiply on DVE.
  - AV: U'[65, 512] += V'_kt.T @ P.T_kt with V' = [V | ones]; row 64
    accumulates L. AV for chunk j is emitted after the score pair of chunk
    j+1 so each pair stays adjacent in the PE stream.
  - o_proj: y tiles [128q, 512] = U'.T chunk @ Wo_h with K=65 (Wo row 64
    zeroed on host so the L row contributes nothing), bf16 out, DMA'd
    unnormalized together with L (bf16).
"""

import sys

for _p in ("/opt/trn_rl_repo", "/root/.axon_site/_ro/trn_rl_repo"):
    if _p not in sys.path:
        sys.path.insert(0, _p)

import numpy as np

import concourse.bass as bass
import concourse.mybir as mybir
import concourse.tile as tile
from concourse import bacc
from concourse.bass_utils import run_bass_kernel_spmd

B = 2
S = 4096
D = 512
H = 8
HD = 64
TOK = B * S          # 8192
NKT = S // 128       # 32 k-tiles per batch
SCALE = HD ** -0.5

F32 = mybir.dt.float32
BF16 = mybir.dt.bfloat16
F8 = mybir.dt.float8e4

_CACHE = {}


def _build():
    nc = bacc.Bacc("TRN2", target_bir_lowering=False, debug=False, num_devices=8)

    xt_d = nc.dram_tensor("xt", [D, TOK], BF16, kind="ExternalInput")
    wqk_d = nc.dram_tensor("wqk", [D, 128], BF16, kind="ExternalInput")
    wv_d = nc.dram_tensor("wv", [D, HD], BF16, kind="ExternalInput")
    wo_d = nc.dram_tensor("wo", [65, D], BF16, kind="ExternalInput")
    bqk_d = nc.dram_tensor("bqk", [128, 1], F32, kind="ExternalInput")
    bv_d = nc.dram_tensor("bv", [HD, 1], F32, kind="ExternalInput")
    mask_d = nc.dram_tensor("mask", [128, 4, 512], BF16, kind="ExternalInput")
    identb_d = nc.dram_tensor("identb", [64, 64], BF16, kind="ExternalInput")
    onesb_d = nc.dram_tensor("onesb", [128, NKT], BF16, kind="ExternalInput")
    y_d = nc.dram_tensor("y", [TOK, D], BF16, kind="ExternalOutput")
    l_d = nc.dram_tensor("l", [TOK], BF16, kind="ExternalOutput")

    xt_r = xt_d.ap().rearrange("(c p) t -> p c t", p=128)      # [128, 4, 8192]
    wqk_r = wqk_d.ap().rearrange("(c p) m -> p c m", p=128)    # [128, 4, 128]
    wv_r = wv_d.ap().rearrange("(c p) m -> p c m", p=128)      # [128, 4, 64]

    with tile.TileContext(nc) as tc:
        import contextlib

        with contextlib.ExitStack() as ctx:
            singles = ctx.enter_context(tc.tile_pool(name="singles", bufs=1))
            xpool = ctx.enter_context(tc.tile_pool(name="xt", bufs=3))
            ptpool = ctx.enter_context(tc.tile_pool(name="pt", bufs=6))
            upool = ctx.enter_context(tc.tile_pool(name="usb", bufs=2))
            ypool = ctx.enter_context(tc.tile_pool(name="ysb", bufs=4))
            kstpool = ctx.enter_context(tc.tile_pool(name="kst", bufs=2))

            ps_st = ctx.enter_context(
                tc.tile_pool(name="ps_st", bufs=2, space="PSUM")
            )
            ps_u = ctx.enter_context(tc.tile_pool(name="ps_u", bufs=2, space="PSUM"))
            ps_misc = ctx.enter_context(
                tc.tile_pool(name="ps_misc", bufs=2, space="PSUM")
            )

            # --- constants / weights -----------------------------------
            wqk_sb = singles.tile([128, 4, 128], BF16)
            wv_sb = singles.tile([128, 4, HD], BF16)
            wo_sb = singles.tile([65, D], BF16)
            bqk_sb = singles.tile([128, 1], F32)
            bv_sb = singles.tile([HD, 1], F32)
            mask_sb = singles.tile([128, 4, 512], BF16)
            identb = singles.tile([64, 64], BF16)
            nc.sync.dma_start(out=wqk_sb, in_=wqk_r)
            nc.sync.dma_start(out=wv_sb, in_=wv_r)
            nc.sync.dma_start(out=wo_sb, in_=wo_d.ap())
            nc.sync.dma_start(out=bqk_sb, in_=bqk_d.ap())
            nc.sync.dma_start(out=bv_sb, in_=bv_d.ap())
            nc.sync.dma_start(out=mask_sb, in_=mask_d.ap())
            nc.sync.dma_start(out=identb, in_=identb_d.ap())

            # --- persistent per-batch activation buffers ---------------
            qt2 = [
                singles.tile([128, S], BF16, tag=f"qt2_{b}", name=f"qt2_{b}")
                for b in range(B)
            ]
            ktp = [
                singles.tile([128, S // 2], BF16, tag=f"ktp_{b}", name=f"ktp_{b}")
                for b in range(B)
            ]
            vp = [
                singles.tile([128, NKT * 65], BF16, tag=f"vp_{b}", name=f"vp_{b}")
                for b in range(B)
            ]
            for b in range(B):
                nc.sync.dma_start(
                    out=vp[b].rearrange("p (t c) -> p t c", c=65)[:, :, 64:65],
                    in_=onesb_d.ap().rearrange("p (t c) -> p t c", c=1),
                )

            def proj_block(b, tb):
                """Projections for 512 tokens (block tb of batch b)."""
                t0 = b * S + tb * 512
                xt_sb = xpool.tile([128, 4, 512], BF16, tag="xt")
                nc.sync.dma_start(out=xt_sb, in_=xt_r[:, :, t0 : t0 + 512])

                qk_ps = ps_misc.tile([128, 512], F32, tag="m")
                for c in range(4):
                    nc.tensor.matmul(
                        qk_ps,
                        wqk_sb[:, c, :],
                        xt_sb[:, c, :],
                        start=(c == 0),
                        stop=(c == 3),
                    )
                vt_ps = ps_misc.tile([HD, 512], F32, tag="m")
                for c in range(4):
                    nc.tensor.matmul(
                        vt_ps,
                        wv_sb[:, c, :],
                        xt_sb[:, c, :],
                        start=(c == 0),
                        stop=(c == 3),
                    )

                # Q.T -> QT2 top half (+bias), then DMA-dup to bottom half
                cols = slice(tb * 512, (tb + 1) * 512)
                nc.vector.tensor_scalar_add(
                    qt2[b][0:64, cols], qk_ps[0:64, :], bqk_sb[0:64, 0:1]
                )
                nc.sync.dma_start(out=qt2[b][64:128, cols], in_=qt2[b][0:64, cols])

                # K.T (+bias) -> ktmp rows 64:128, then DMA-scatter into the
                # even/odd packed layout of ktp (pair (2j, 2j+1) shares cols)
                ktmp = kstpool.tile([128, 512], BF16, tag="ktmp")
                nc.vector.tensor_scalar_add(
                    ktmp[64:128, :], qk_ps[64:128, :], bqk_sb[64:128, 0:1]
                )
                ksrc = ktmp[64:128, :].rearrange("p (a b c) -> p a b c", b=2, c=128)
                kdst = ktp[b][:, tb * 256 : (tb + 1) * 256].rearrange(
                    "p (a c) -> p a c", c=128
                )
                nc.sync.dma_start(out=kdst[0:64], in_=ksrc[:, :, 0, :])
                nc.sync.dma_start(out=kdst[64:128], in_=ksrc[:, :, 1, :])

                # V.T (+bias, bf16) -> PE transpose to V natural -> V' blocks
                vt_sb = kstpool.tile([HD, 512], BF16, tag="vt")
                nc.vector.tensor_scalar_add(vt_sb, vt_ps, bv_sb[:, 0:1])
                for j in range(4):
                    kt = tb * 4 + j
                    vtr_ps = ps_misc.tile([128, HD], BF16, tag="m")
                    nc.tensor.transpose(
                        vtr_ps, vt_sb[:, j * 128 : (j + 1) * 128], identb
                    )
                    nc.vector.tensor_copy(vp[b][:, kt * 65 : kt * 65 + 64], vtr_ps)

            def attn_qblock(b, qb):
                """Attention + unnormalized o_proj for q-block qb of batch b."""
                q0 = qb * 512
                u_ps = ps_u.tile([65, 512], F32, tag="u")
                n_chunks = 2 * (qb + 1)  # chunks of 2 k-tiles

                def emit_av(pt, j, w):
                    for j2 in range(2):
                        kt = 2 * j + j2
                        nc.tensor.matmul(
                            u_ps[:, 512 - w : 512],
                            vp[b][:, kt * 65 : kt * 65 + 65],
                            pt[:, j2, 0:w],
                            start=(kt == 0),
                            stop=(kt == 2 * n_chunks - 1),
                            skip_group_check=True,
                        )

                # Super-chunks of 2 chunks: the 4 score matmuls (2 packed
                # K=64 pairs) are emitted back-to-back so the PE stays in
                # 64-row tiling mode for the whole burst (mode switches
                # drain the array and defeat pair concurrency); the 128-mode
                # AV matmuls of the previous super-chunk follow the burst.
                # The last (diagonal) chunk covers keys the first 256
                # queries never see: narrow it to the top 256 queries.
                prev = []
                for j2 in range(n_chunks // 2):
                    pts = []
                    sts = []
                    for j in (2 * j2, 2 * j2 + 1):
                        w = 256 if j == n_chunks - 1 else 512
                        qcols = slice(q0 + 512 - w, q0 + 512)
                        st = ps_st.tile([128, 2, 512], F32, tag="st")
                        kcols = slice(j * 128, (j + 1) * 128)
                        nc.tensor.matmul(
                            st[:, 0, 0:w],
                            ktp[b][0:64, kcols],
                            qt2[b][0:64, qcols],
                            start=True,
                            stop=True,
                        )
                        nc.tensor.matmul(
                            st[:, 1, 0:w],
                            ktp[b][64:128, kcols],
                            qt2[b][64:128, qcols],
                            start=True,
                            stop=True,
                        )
                        sts.append((st, w))
                    for j, (st, w) in zip((2 * j2, 2 * j2 + 1), sts):
                        pt = ptpool.tile([128, 2, 512], BF16, tag="pt")
                        for jb in range(2):
                            nc.scalar.activation(
                                pt[:, jb, 0:w],
                                st[:, jb, 0:w],
                                mybir.ActivationFunctionType.Exp,
                                scale=SCALE,
                            )
                        if j == n_chunks - 2:  # diagonal, full width
                            nc.vector.tensor_mul(
                                pt, pt, mask_sb[:, 0:2, :]
                            )
                        elif j == n_chunks - 1:  # diagonal, narrowed
                            nc.vector.tensor_mul(
                                pt[:, :, 0:256],
                                pt[:, :, 0:256],
                                mask_sb[:, 0:2, 0:256],
                            )
                        pts.append((pt, j, w))
                    for pt, j, w in prev:
                        emit_av(pt, j, w)
                    prev = pts
                for pt, j, w in prev:
                    emit_av(pt, j, w)

                # U' -> SBUF [65, 512] bf16 (U rows 0:64, L row 64)
                u_sb = upool.tile([65, 512], BF16, tag="u")
                nc.vector.tensor_copy(u_sb, u_ps)

                row0 = b * S + q0
                nc.sync.dma_start(
                    out=l_d.ap()[row0 : row0 + 512].rearrange("(p c) -> p c", p=1),
                    in_=u_sb[64:65, :],
                )

                # y = U'.T @ Wo_h (unnormalized); K=65 with wo row 64 = 0
                for j2 in range(4):
                    y_ps = ps_misc.tile([128, 512], F32, tag="m")
                    nc.tensor.matmul(
                        y_ps,
                        u_sb[:, j2 * 128 : (j2 + 1) * 128],
                        wo_sb,
                        start=True,
                        stop=True,
                    )
                    y_sb = ypool.tile([128, 512], BF16, tag="y")
                    nc.vector.tensor_copy(y_sb, y_ps)
                    r0 = row0 + j2 * 128
                    nc.sync.dma_start(out=y_d.ap()[r0 : r0 + 128, :], in_=y_sb)

            # Pipeline: proj(tb) immediately enables attn(qb=tb).
            for b in range(B):
                for tb in range(8):
                    proj_block(b, tb)
                    attn_qblock(b, tb)

    nc.compile()
    return nc


def _prep_inputs(x, Wq, bq, Wk, bk, Wv, bv, Wo, bo):
    import ml_dtypes

    bf = ml_dtypes.bfloat16
    xt = np.ascontiguousarray(x.reshape(TOK, D).T).astype(bf)
    mask = np.zeros((128, 4, 512), dtype=np.float32)
    p = np.arange(128)[:, None]
    c = np.arange(512)[None, :]
    for d in range(4):
        mask[:, d, :] = (p + 128 * d <= c).astype(np.float32)
    mask = mask.astype(bf)
    identb = np.eye(64, dtype=np.float32).astype(bf)
    onesb = np.ones((128, NKT), dtype=np.float32).astype(bf)

    in_maps = []
    for h in range(H):
        hs = slice(h * HD, (h + 1) * HD)
        wo_h = np.concatenate(
            [Wo[hs, :], np.zeros((1, D), dtype=np.float32)], axis=0
        )
        in_maps.append(
            {
                "xt": xt,
                "wqk": np.ascontiguousarray(
                    np.concatenate([Wq[:, hs], Wk[:, hs]], axis=1)
                ).astype(bf),
                "wv": np.ascontiguousarray(Wv[:, hs]).astype(bf),
                "wo": wo_h.astype(bf),
                "bqk": np.concatenate([bq[hs], bk[hs]]).reshape(128, 1).astype(
                    np.float32
                ),
                "bv": bv[hs].reshape(HD, 1).astype(np.float32),
                "mask": mask,
                "identb": identb,
                "onesb": onesb,
            }
        )
    return in_maps


def _install_ntff_hook():
    """Register the axon NTFF profiling hook (test-only plumbing)."""
    import types

    try:
        from antenv.axon_hooks import set_axon_ntff_profile_hook  # noqa: F401
    except ImportError:
        m = types.ModuleType("antenv.axon_hooks")
        m._HOOK = None
        m.set_axon_ntff_profile_hook = lambda h: setattr(m, "_HOOK", h)
        m.get_axon_ntff_profile_hook = lambda: m._HOOK
        sys.modules["antenv.axon_hooks"] = m
        import antenv

        antenv.axon_hooks = m
    from antenv.axon_hooks import (
        get_axon_ntff_profile_hook,
        set_axon_ntff_profile_hook,
    )

    if get_axon_ntff_profile_hook() is None:
        import trn_agent_boot.trn_boot as tb

        set_axon_ntff_profile_hook(
            tb._ntff_profile_via_ctypes("/opt/axon/libaxon_pjrt.so")
        )


def kernel(x, Wq, bq, Wk, bk, Wv, bv, Wo, bo, _trace=False):
    x, Wq, bq, Wk, bk, Wv, bv, Wo, bo = (
        np.asarray(a, dtype=np.float32) for a in (x, Wq, bq, Wk, bk, Wv, bv, Wo, bo)
    )
    if "nc" not in _CACHE:
        _CACHE["nc"] = _build()
    nc = _CACHE["nc"]
    in_maps = _prep_inputs(x, Wq, bq, Wk, bk, Wv, bv, Wo, bo)
    kwargs = {}
    if _trace:
        _install_ntff_hook()
        kwargs = dict(trace=True, trace_cores=[0])
    res = run_bass_kernel_spmd(nc, in_maps, core_ids=list(range(8)), **kwargs)
    _CACHE["last_result"] = res
    y = np.zeros((TOK, D), dtype=np.float64)
    for r in res.results:
        y += r["y"].astype(np.float64) / r["l"].astype(np.float64)[:, None]
    y += bo[None, :]
    return y.astype(np.float32).reshape(B, S, D)


# revision 21
# speedup vs baseline: 1.1949x; 1.1816x over previous
"""Causal self-attention (B=2, S=4096, D=512, H=8) on 8 Trainium2 NeuronCores.

Sharding: tensor-parallel over heads. Core h computes head h for both batch
elements: QKV projections for its head, causal flash attention, and its
partial (unnormalized) o_proj contribution y_h = U_h @ Wo[h*64:(h+1)*64, :]
plus the per-query softmax denominators L_h. The host computes
sum_h(y_h / L_h) + bo.

All matmuls run in bf16 (fp32 PSUM accumulation). Score matmuls have K=64
(head dim), so consecutive k-tiles are packed onto the two 64-row groups of
the PE array via tile_position (auto-derived from base partitions) and run
concurrently:
  - qt2 [128, 4096]/batch: Q.T duplicated in partition halves 0:64 / 64:128.
  - ktp [128, 2048]/batch: even k-tiles' K.T in partitions 0:64, odd in
    64:128; column block j*128:(j+1)*128 holds the pair (2j, 2j+1).
  - chunk j: two concurrent matmuls -> st psum [128, 2, 512] (two banks),
    one ACT exp [128, 1024] PSUM->SBUF -> P.T bf16; diagonal chunks get a
    0/1 causal mask multiply on DVE.
  - AV: U'[65, 512] += V'_kt.T @ P.T_kt with V' = [V | ones]; row 64
    accumulates L. AV for chunk j is emitted after the score pair of chunk
    j+1 so each pair stays adjacent in the PE stream.
  - o_proj: y tiles [128q, 512] = U'.T chunk @ Wo_h with K=65 (Wo row 64
    zeroed on host so the L row contributes nothing), bf16 out, DMA'd
    unnormalized together with L (bf16).
"""

import sys

for _p in ("/opt/trn_rl_repo", "/root/.axon_site/_ro/trn_rl_repo"):
    if _p not in sys.path:
        sys.path.insert(0, _p)

import numpy as np

import concourse.bass as bass
import concourse.mybir as mybir
import concourse.tile as tile
from concourse import bacc
from concourse.bass_utils import run_bass_kernel_spmd

B = 2
S = 4096
D = 512
H = 8
HD = 64
TOK = B * S          # 8192
NKT = S // 128       # 32 k-tiles per batch
SCALE = HD ** -0.5

F32 = mybir.dt.float32
BF16 = mybir.dt.bfloat16
F8 = mybir.dt.float8e4

_CACHE = {}


def _build():
    nc = bacc.Bacc("TRN2", target_bir_lowering=False, debug=False, num_devices=8)

    xt_d = nc.dram_tensor("xt", [D, TOK], BF16, kind="ExternalInput")
    wqk_d = nc.dram_tensor("wqk", [D, 128], BF16, kind="ExternalInput")
    wv_d = nc.dram_tensor("wv", [D, HD], BF16, kind="ExternalInput")
    wo_d = nc.dram_tensor("wo", [65, D], BF16, kind="ExternalInput")
    bqk_d = nc.dram_tensor("bqk", [128, 1], F32, kind="ExternalInput")
    bv_d = nc.dram_tensor("bv", [HD, 1], F32, kind="ExternalInput")
    mask_d = nc.dram_tensor("mask", [128, 4, 512], BF16, kind="ExternalInput")
    identb_d = nc.dram_tensor("identb", [64, 64], BF16, kind="ExternalInput")
    onesb_d = nc.dram_tensor("onesb", [128, NKT], BF16, kind="ExternalInput")
    y_d = nc.dram_tensor("y", [TOK, D], BF16, kind="ExternalOutput")
    l_d = nc.dram_tensor("l", [TOK], BF16, kind="ExternalOutput")

    xt_r = xt_d.ap().rearrange("(c p) t -> p c t", p=128)      # [128, 4, 8192]
    wqk_r = wqk_d.ap().rearrange("(c p) m -> p c m", p=128)    # [128, 4, 128]
    wv_r = wv_d.ap().rearrange("(c p) m -> p c m", p=128)      # [128, 4, 64]

    with tile.TileContext(nc) as tc:
        import contextlib

        with contextlib.ExitStack() as ctx:
            singles = ctx.enter_context(tc.tile_pool(name="singles", bufs=1))
            xpool = ctx.enter_context(tc.tile_pool(name="xt", bufs=3))
            ptpool = ctx.enter_context(tc.tile_pool(name="pt", bufs=6))
            upool = ctx.enter_context(tc.tile_pool(name="usb", bufs=2))
            ypool = ctx.enter_context(tc.tile_pool(name="ysb", bufs=4))
            kstpool = ctx.enter_context(tc.tile_pool(name="kst", bufs=2))

            ps_st = ctx.enter_context(
                tc.tile_pool(name="ps_st", bufs=2, space="PSUM")
            )
            ps_u = ctx.enter_context(tc.tile_pool(name="ps_u", bufs=2, space="PSUM"))
            ps_misc = ctx.enter_context(
                tc.tile_pool(name="ps_misc", bufs=2, space="PSUM")
            )

            # --- constants / weights -----------------------------------
            wqk_sb = singles.tile([128, 4, 128], BF16)
            wv_sb = singles.tile([128, 4, HD], BF16)
            wo_sb = singles.tile([65, D], BF16)
            bqk_sb = singles.tile([128, 1], F32)
            bv_sb = singles.tile([HD, 1], F32)
            mask_sb = singles.tile([128, 4, 512], BF16)
            identb = singles.tile([64, 64], BF16)
            nc.sync.dma_start(out=wqk_sb, in_=wqk_r)
            nc.sync.dma_start(out=wv_sb, in_=wv_r)
            nc.sync.dma_start(out=wo_sb, in_=wo_d.ap())
            nc.sync.dma_start(out=bqk_sb, in_=bqk_d.ap())
            nc.sync.dma_start(out=bv_sb, in_=bv_d.ap())
            nc.sync.dma_start(out=mask_sb, in_=mask_d.ap())
            nc.sync.dma_start(out=identb, in_=identb_d.ap())

            # --- persistent per-batch activation buffers ---------------
            qt2 = [
                singles.tile([128, S], BF16, tag=f"qt2_{b}", name=f"qt2_{b}")
                for b in range(B)
            ]
            ktp = [
                singles.tile([128, S // 2], BF16, tag=f"ktp_{b}", name=f"ktp_{b}")
                for b in range(B)
            ]
            vp = [
                singles.tile([128, NKT * 65], BF16, tag=f"vp_{b}", name=f"vp_{b}")
                for b in range(B)
            ]
            for b in range(B):
                nc.sync.dma_start(
                    out=vp[b].rearrange("p (t c) -> p t c", c=65)[:, :, 64:65],
                    in_=onesb_d.ap().rearrange("p (t c) -> p t c", c=1),
                )

            def proj_block(b, tb):
                """Projections for 512 tokens (block tb of batch b)."""
                t0 = b * S + tb * 512
                xt_sb = xpool.tile([128, 4, 512], BF16, tag="xt")
                nc.sync.dma_start(out=xt_sb, in_=xt_r[:, :, t0 : t0 + 512])

                qk_ps = ps_misc.tile([128, 512], F32, tag="m")
                for c in range(4):
                    nc.tensor.matmul(
                        qk_ps,
                        wqk_sb[:, c, :],
                        xt_sb[:, c, :],
                        start=(c == 0),
                        stop=(c == 3),
                    )
                vt_ps = ps_misc.tile([HD, 512], F32, tag="m")
                for c in range(4):
                    nc.tensor.matmul(
                        vt_ps,
                        wv_sb[:, c, :],
                        xt_sb[:, c, :],
                        start=(c == 0),
                        stop=(c == 3),
                    )

                # Q.T -> both QT2 halves (+bias) via two DVE adds (the
                # second write is cross-partition-base; avoids the dup DMA
                # latency gating each q-block's first score burst)
                cols = slice(tb * 512, (tb + 1) * 512)
                nc.vector.tensor_scalar_add(
                    qt2[b][0:64, cols], qk_ps[0:64, :], bqk_sb[0:64, 0:1]
                )
                nc.vector.tensor_scalar_add(
                    qt2[b][64:128, cols], qk_ps[0:64, :], bqk_sb[0:64, 0:1]
                )

                # K.T (+bias) -> ktmp rows 64:128, then DMA-scatter into the
                # even/odd packed layout of ktp (pair (2j, 2j+1) shares cols)
                ktmp = kstpool.tile([128, 512], BF16, tag="ktmp")
                nc.vector.tensor_scalar_add(
                    ktmp[64:128, :], qk_ps[64:128, :], bqk_sb[64:128, 0:1]
                )
                ksrc = ktmp[64:128, :].rearrange("p (a b c) -> p a b c", b=2, c=128)
                kdst = ktp[b][:, tb * 256 : (tb + 1) * 256].rearrange(
                    "p (a c) -> p a c", c=128
                )
                nc.sync.dma_start(out=kdst[0:64], in_=ksrc[:, :, 0, :])
                nc.sync.dma_start(out=kdst[64:128], in_=ksrc[:, :, 1, :])

                # V.T (+bias, bf16) -> PE transpose to V natural -> V' blocks
                vt_sb = kstpool.tile([HD, 512], BF16, tag="vt")
                nc.vector.tensor_scalar_add(vt_sb, vt_ps, bv_sb[:, 0:1])
                for j in range(4):
                    kt = tb * 4 + j
                    vtr_ps = ps_misc.tile([128, HD], BF16, tag="m")
                    nc.tensor.transpose(
                        vtr_ps, vt_sb[:, j * 128 : (j + 1) * 128], identb
                    )
                    nc.vector.tensor_copy(vp[b][:, kt * 65 : kt * 65 + 64], vtr_ps)

            def attn_qblock(b, qb):
                """Attention + unnormalized o_proj for q-block qb of batch b."""
                q0 = qb * 512
                u_ps = ps_u.tile([65, 512], F32, tag="u")
                n_chunks = 2 * (qb + 1)  # chunks of 2 k-tiles

                def emit_av(pt, j, w):
                    for j2 in range(2):
                        kt = 2 * j + j2
                        nc.tensor.matmul(
                            u_ps[:, 512 - w : 512],
                            vp[b][:, kt * 65 : kt * 65 + 65],
                            pt[:, j2, 0:w],
                            start=(kt == 0),
                            stop=(kt == 2 * n_chunks - 1),
                            skip_group_check=True,
                        )

                # Super-chunks of 2 chunks: the 4 score matmuls (2 packed
                # K=64 pairs) are emitted back-to-back so the PE stays in
                # 64-row tiling mode for the whole burst (mode switches
                # drain the array and defeat pair concurrency); the 128-mode
                # AV matmuls of the previous super-chunk follow the burst.
                # The last (diagonal) chunk covers keys the first 256
                # queries never see: narrow it to the top 256 queries.
                prev = []
                for j2 in range(n_chunks // 2):
                    pts = []
                    sts = []
                    for j in (2 * j2, 2 * j2 + 1):
                        w = 256 if j == n_chunks - 1 else 512
                        qcols = slice(q0 + 512 - w, q0 + 512)
                        st = ps_st.tile([128, 2, 512], F32, tag="st")
                        kcols = slice(j * 128, (j + 1) * 128)
                        nc.tensor.matmul(
                            st[:, 0, 0:w],
                            ktp[b][0:64, kcols],
                            qt2[b][0:64, qcols],
                            start=True,
                            stop=True,
                        )
                        nc.tensor.matmul(
                            st[:, 1, 0:w],
                            ktp[b][64:128, kcols],
                            qt2[b][64:128, qcols],
                            start=True,
                            stop=True,
                        )
                        sts.append((st, w))
                    for j, (st, w) in zip((2 * j2, 2 * j2 + 1), sts):
                        pt = ptpool.tile([128, 2, 512], BF16, tag="pt")
                        nc.scalar.activation(
                            pt[:, :, 0:w],
                            st[:, :, 0:w],
                            mybir.ActivationFunctionType.Exp,
                            scale=SCALE,
                        )
                        if j == n_chunks - 2:  # diagonal, full width
                            nc.vector.tensor_mul(
                                pt, pt, mask_sb[:, 0:2, :]
                            )
                        elif j == n_chunks - 1:  # diagonal, narrowed
                            nc.vector.tensor_mul(
                                pt[:, :, 0:256],
                                pt[:, :, 0:256],
                                mask_sb[:, 0:2, 0:256],
                            )
                        pts.append((pt, j, w))
                    for pt, j, w in prev:
                        emit_av(pt, j, w)
                    prev = pts
                for pt, j, w in prev:
                    emit_av(pt, j, w)

                # U' -> SBUF [65, 512] bf16 (U rows 0:64, L row 64)
                u_sb = upool.tile([65, 512], BF16, tag="u")
                nc.vector.tensor_copy(u_sb, u_ps)

                row0 = b * S + q0
                nc.sync.dma_start(
                    out=l_d.ap()[row0 : row0 + 512].rearrange("(p c) -> p c", p=1),
                    in_=u_sb[64:65, :],
                )

                # y = U'.T @ Wo_h (unnormalized); K=65 with wo row 64 = 0
                for j2 in range(4):
                    y_ps = ps_misc.tile([128, 512], F32, tag="m")
                    nc.tensor.matmul(
                        y_ps,
                        u_sb[:, j2 * 128 : (j2 + 1) * 128],
                        wo_sb,
                        start=True,
                        stop=True,
                    )
                    y_sb = ypool.tile([128, 512], BF16, tag="y")
                    nc.vector.tensor_copy(y_sb, y_ps)
                    r0 = row0 + j2 * 128
                    nc.sync.dma_start(out=y_d.ap()[r0 : r0 + 128, :], in_=y_sb)

            # Pipeline: proj(tb) immediately enables attn(qb=tb).
            for b in range(B):
                for tb in range(8):
                    proj_block(b, tb)
                    attn_qblock(b, tb)

    nc.compile()
    return nc


def _prep_inputs(x, Wq, bq, Wk, bk, Wv, bv, Wo, bo):
    import ml_dtypes

    bf = ml_dtypes.bfloat16
    xt = np.ascontiguousarray(x.reshape(TOK, D).T).astype(bf)
    mask = np.zeros((128, 4, 512), dtype=np.float32)
    p = np.arange(128)[:, None]
    c = np.arange(512)[None, :]
    for d in range(4):
        mask[:, d, :] = (p + 128 * d <= c).astype(np.float32)
    mask = mask.astype(bf)
    identb = np.eye(64, dtype=np.float32).astype(bf)
    onesb = np.ones((128, NKT), dtype=np.float32).astype(bf)

    in_maps = []
    for h in range(H):
        hs = slice(h * HD, (h + 1) * HD)
        wo_h = np.concatenate(
            [Wo[hs, :], np.zeros((1, D), dtype=np.float32)], axis=0
        )
        in_maps.append(
            {
                "xt": xt,
                "wqk": np.ascontiguousarray(
                    np.concatenate([Wq[:, hs], Wk[:, hs]], axis=1)
                ).astype(bf),
                "wv": np.ascontiguousarray(Wv[:, hs]).astype(bf),
                "wo": wo_h.astype(bf),
                "bqk": np.concatenate([bq[hs], bk[hs]]).reshape(128, 1).astype(
                    np.float32
                ),
                "bv": bv[hs].reshape(HD, 1).astype(np.float32),
                "mask": mask,
                "identb": identb,
                "onesb": onesb,
            }
        )
    return in_maps


def _install_ntff_hook():
    """Register the axon NTFF profiling hook (test-only plumbing)."""
    import types

    try:
        from antenv.axon_hooks import set_axon_ntff_profile_hook  # noqa: F401
    except ImportError:
        m = types.ModuleType("antenv.axon_hooks")
        m._HOOK = None
        m.set_axon_ntff_profile_hook = lambda h: setattr(m, "_HOOK", h)
        m.get_axon_ntff_profile_hook = lambda: m._HOOK
        sys.modules["antenv.axon_hooks"] = m
        import antenv

        antenv.axon_hooks = m
    from antenv.axon_hooks import (
        get_axon_ntff_profile_hook,
        set_axon_ntff_profile_hook,
    )

    if get_axon_ntff_profile_hook() is None:
        import trn_agent_boot.trn_boot as tb

        set_axon_ntff_profile_hook(
            tb._ntff_profile_via_ctypes("/opt/axon/libaxon_pjrt.so")
        )


def kernel(x, Wq, bq, Wk, bk, Wv, bv, Wo, bo, _trace=False):
    x, Wq, bq, Wk, bk, Wv, bv, Wo, bo = (
        np.asarray(a, dtype=np.float32) for a in (x, Wq, bq, Wk, bk, Wv, bv, Wo, bo)
    )
    if "nc" not in _CACHE:
        _CACHE["nc"] = _build()
    nc = _CACHE["nc"]
    in_maps = _prep_inputs(x, Wq, bq, Wk, bk, Wv, bv, Wo, bo)
    kwargs = {}
    if _trace:
        _install_ntff_hook()
        kwargs = dict(trace=True, trace_cores=[0])
    res = run_bass_kernel_spmd(nc, in_maps, core_ids=list(range(8)), **kwargs)
    _CACHE["last_result"] = res
    y = np.zeros((TOK, D), dtype=np.float64)
    for r in res.results:
        y += r["y"].astype(np.float64) / r["l"].astype(np.float64)[:, None]
    y += bo[None, :]
    return y.astype(np.float32).reshape(B, S, D)


# revision 22
# speedup vs baseline: 1.2479x; 1.0443x over previous
"""Causal self-attention (B=2, S=4096, D=512, H=8) on 8 Trainium2 NeuronCores.

Sharding: tensor-parallel over heads. Core h computes head h for both batch
elements: QKV projections for its head, causal flash attention, and its
partial (unnormalized) o_proj contribution y_h = U_h @ Wo[h*64:(h+1)*64, :]
plus the per-query softmax denominators L_h. The host computes
sum_h(y_h / L_h) + bo.

All matmuls run in bf16 (fp32 PSUM accumulation). Score matmuls have K=64
(head dim), so consecutive k-tiles are packed onto the two 64-row groups of
the PE array via tile_position (auto-derived from base partitions) and run
concurrently:
  - qt2 [128, 4096]/batch: Q.T duplicated in partition halves 0:64 / 64:128.
  - ktp [128, 2048]/batch: even k-tiles' K.T in partitions 0:64, odd in
    64:128; column block j*128:(j+1)*128 holds the pair (2j, 2j+1).
  - chunk j: two concurrent matmuls -> st psum [128, 2, 512] (two banks),
    one ACT exp [128, 1024] PSUM->SBUF -> P.T bf16; diagonal chunks get a
    0/1 causal mask multiply on DVE.
  - AV: U'[65, 512] += V'_kt.T @ P.T_kt with V' = [V | ones]; row 64
    accumulates L. AV for chunk j is emitted after the score pair of chunk
    j+1 so each pair stays adjacent in the PE stream.
  - o_proj: y tiles [128q, 512] = U'.T chunk @ Wo_h with K=65 (Wo row 64
    zeroed on host so the L row contributes nothing), bf16 out, DMA'd
    unnormalized together with L (bf16).
"""

import sys

for _p in ("/opt/trn_rl_repo", "/root/.axon_site/_ro/trn_rl_repo"):
    if _p not in sys.path:
        sys.path.insert(0, _p)

import numpy as np

import concourse.bass as bass
import concourse.mybir as mybir
import concourse.tile as tile
from concourse import bacc
from concourse.bass_utils import run_bass_kernel_spmd

B = 2
S = 4096
D = 512
H = 8
HD = 64
TOK = B * S          # 8192
NKT = S // 128       # 32 k-tiles per batch
SCALE = HD ** -0.5

F32 = mybir.dt.float32
BF16 = mybir.dt.bfloat16
F8 = mybir.dt.float8e4

_CACHE = {}


def _build():
    nc = bacc.Bacc("TRN2", target_bir_lowering=False, debug=False, num_devices=8)

    xt_d = nc.dram_tensor("xt", [D, TOK], BF16, kind="ExternalInput")
    wqk_d = nc.dram_tensor("wqk", [D, 128], BF16, kind="ExternalInput")
    wv_d = nc.dram_tensor("wv", [D, HD], BF16, kind="ExternalInput")
    wo_d = nc.dram_tensor("wo", [65, D], BF16, kind="ExternalInput")
    bqk_d = nc.dram_tensor("bqk", [128, 1], F32, kind="ExternalInput")
    bv_d = nc.dram_tensor("bv", [HD, 1], F32, kind="ExternalInput")
    mask_d = nc.dram_tensor("mask", [128, 4, 512], BF16, kind="ExternalInput")
    identb_d = nc.dram_tensor("identb", [64, 64], BF16, kind="ExternalInput")
    onesb_d = nc.dram_tensor("onesb", [128, NKT], BF16, kind="ExternalInput")
    y_d = nc.dram_tensor("y", [TOK, D], BF16, kind="ExternalOutput")
    l_d = nc.dram_tensor("l", [TOK], BF16, kind="ExternalOutput")

    xt_r = xt_d.ap().rearrange("(c p) t -> p c t", p=128)      # [128, 4, 8192]
    wqk_r = wqk_d.ap().rearrange("(c p) m -> p c m", p=128)    # [128, 4, 128]
    wv_r = wv_d.ap().rearrange("(c p) m -> p c m", p=128)      # [128, 4, 64]

    with tile.TileContext(nc) as tc:
        import contextlib

        with contextlib.ExitStack() as ctx:
            singles = ctx.enter_context(tc.tile_pool(name="singles", bufs=1))
            xpool = ctx.enter_context(tc.tile_pool(name="xt", bufs=3))
            ptpool = ctx.enter_context(tc.tile_pool(name="pt", bufs=6))
            upool = ctx.enter_context(tc.tile_pool(name="usb", bufs=2))
            ypool = ctx.enter_context(tc.tile_pool(name="ysb", bufs=4))
            kstpool = ctx.enter_context(tc.tile_pool(name="kst", bufs=2))

            ps_st = ctx.enter_context(
                tc.tile_pool(name="ps_st", bufs=2, space="PSUM")
            )
            ps_u = ctx.enter_context(tc.tile_pool(name="ps_u", bufs=2, space="PSUM"))
            ps_misc = ctx.enter_context(
                tc.tile_pool(name="ps_misc", bufs=2, space="PSUM")
            )

            # --- constants / weights -----------------------------------
            wqk_sb = singles.tile([128, 4, 128], BF16)
            wv_sb = singles.tile([128, 4, HD], BF16)
            wo_sb = singles.tile([65, D], BF16)
            bqk_sb = singles.tile([128, 1], F32)
            bv_sb = singles.tile([HD, 1], F32)
            mask_sb = singles.tile([128, 4, 512], BF16)
            identb = singles.tile([64, 64], BF16)
            nc.sync.dma_start(out=wqk_sb, in_=wqk_r)
            nc.sync.dma_start(out=wv_sb, in_=wv_r)
            nc.sync.dma_start(out=wo_sb, in_=wo_d.ap())
            nc.sync.dma_start(out=bqk_sb, in_=bqk_d.ap())
            nc.sync.dma_start(out=bv_sb, in_=bv_d.ap())
            nc.sync.dma_start(out=mask_sb, in_=mask_d.ap())
            nc.sync.dma_start(out=identb, in_=identb_d.ap())

            # --- persistent per-batch activation buffers ---------------
            qt2 = [
                singles.tile([128, S], BF16, tag=f"qt2_{b}", name=f"qt2_{b}")
                for b in range(B)
            ]
            ktp = [
                singles.tile([128, S // 2], BF16, tag=f"ktp_{b}", name=f"ktp_{b}")
                for b in range(B)
            ]
            vp = [
                singles.tile([128, NKT * 65], BF16, tag=f"vp_{b}", name=f"vp_{b}")
                for b in range(B)
            ]
            for b in range(B):
                nc.sync.dma_start(
                    out=vp[b].rearrange("p (t c) -> p t c", c=65)[:, :, 64:65],
                    in_=onesb_d.ap().rearrange("p (t c) -> p t c", c=1),
                )

            def proj_block(b, tb):
                """Projections for 512 tokens (block tb of batch b)."""
                t0 = b * S + tb * 512
                xt_sb = xpool.tile([128, 4, 512], BF16, tag="xt")
                nc.sync.dma_start(out=xt_sb, in_=xt_r[:, :, t0 : t0 + 512])

                qk_ps = ps_misc.tile([128, 512], F32, tag="m")
                for c in range(4):
                    nc.tensor.matmul(
                        qk_ps,
                        wqk_sb[:, c, :],
                        xt_sb[:, c, :],
                        start=(c == 0),
                        stop=(c == 3),
                    )
                vt_ps = ps_misc.tile([HD, 512], F32, tag="m")
                for c in range(4):
                    nc.tensor.matmul(
                        vt_ps,
                        wv_sb[:, c, :],
                        xt_sb[:, c, :],
                        start=(c == 0),
                        stop=(c == 3),
                    )

                # Q.T -> both QT2 halves (+bias) via two DVE adds (the
                # second write is cross-partition-base; avoids the dup DMA
                # latency gating each q-block's first score burst)
                cols = slice(tb * 512, (tb + 1) * 512)
                nc.vector.tensor_scalar_add(
                    qt2[b][0:64, cols], qk_ps[0:64, :], bqk_sb[0:64, 0:1]
                )
                nc.vector.tensor_scalar_add(
                    qt2[b][64:128, cols], qk_ps[0:64, :], bqk_sb[0:64, 0:1]
                )

                # K.T (+bias) scattered straight from PSUM into the
                # even/odd packed ktp layout via two DVE adds (even k-tiles
                # cross-partition-base 64->0; odd stay at 64)
                ksrc = qk_ps[64:128, :].rearrange("p (a b c) -> p a b c", b=2, c=128)
                kdst = ktp[b][:, tb * 256 : (tb + 1) * 256].rearrange(
                    "p (a c) -> p a c", c=128
                )
                nc.vector.tensor_scalar_add(
                    kdst[0:64], ksrc[:, :, 0, :], bqk_sb[64:128, 0:1]
                )
                nc.vector.tensor_scalar_add(
                    kdst[64:128], ksrc[:, :, 1, :], bqk_sb[64:128, 0:1]
                )

                # V.T (+bias, bf16) -> PE transpose to V natural -> V' blocks
                vt_sb = kstpool.tile([HD, 512], BF16, tag="vt")
                nc.vector.tensor_scalar_add(vt_sb, vt_ps, bv_sb[:, 0:1])
                for j in range(4):
                    kt = tb * 4 + j
                    vtr_ps = ps_misc.tile([128, HD], BF16, tag="m")
                    nc.tensor.transpose(
                        vtr_ps, vt_sb[:, j * 128 : (j + 1) * 128], identb
                    )
                    nc.vector.tensor_copy(vp[b][:, kt * 65 : kt * 65 + 64], vtr_ps)

            def attn_qblock(b, qb):
                """Attention + unnormalized o_proj for q-block qb of batch b."""
                q0 = qb * 512
                u_ps = ps_u.tile([65, 512], F32, tag="u")
                n_chunks = 2 * (qb + 1)  # chunks of 2 k-tiles

                def emit_av(pt, j, w):
                    for j2 in range(2):
                        kt = 2 * j + j2
                        nc.tensor.matmul(
                            u_ps[:, 512 - w : 512],
                            vp[b][:, kt * 65 : kt * 65 + 65],
                            pt[:, j2, 0:w],
                            start=(kt == 0),
                            stop=(kt == 2 * n_chunks - 1),
                            skip_group_check=True,
                        )

                # Super-chunks of 2 chunks: the 4 score matmuls (2 packed
                # K=64 pairs) are emitted back-to-back so the PE stays in
                # 64-row tiling mode for the whole burst (mode switches
                # drain the array and defeat pair concurrency); the 128-mode
                # AV matmuls of the previous super-chunk follow the burst.
                # The last (diagonal) chunk covers keys the first 256
                # queries never see: narrow it to the top 256 queries.
                prev = []
                for j2 in range(n_chunks // 2):
                    pts = []
                    sts = []
                    for j in (2 * j2, 2 * j2 + 1):
                        w = 256 if j == n_chunks - 1 else 512
                        qcols = slice(q0 + 512 - w, q0 + 512)
                        st = ps_st.tile([128, 2, 512], F32, tag="st")
                        kcols = slice(j * 128, (j + 1) * 128)
                        nc.tensor.matmul(
                            st[:, 0, 0:w],
                            ktp[b][0:64, kcols],
                            qt2[b][0:64, qcols],
                            start=True,
                            stop=True,
                        )
                        nc.tensor.matmul(
                            st[:, 1, 0:w],
                            ktp[b][64:128, kcols],
                            qt2[b][64:128, qcols],
                            start=True,
                            stop=True,
                        )
                        sts.append((st, w))
                    for j, (st, w) in zip((2 * j2, 2 * j2 + 1), sts):
                        pt = ptpool.tile([128, 2, 512], BF16, tag="pt")
                        nc.scalar.activation(
                            pt[:, :, 0:w],
                            st[:, :, 0:w],
                            mybir.ActivationFunctionType.Exp,
                            scale=SCALE,
                        )
                        if j == n_chunks - 2:  # diagonal, full width
                            nc.vector.tensor_mul(
                                pt, pt, mask_sb[:, 0:2, :]
                            )
                        elif j == n_chunks - 1:  # diagonal, narrowed
                            nc.vector.tensor_mul(
                                pt[:, :, 0:256],
                                pt[:, :, 0:256],
                                mask_sb[:, 0:2, 0:256],
                            )
                        pts.append((pt, j, w))
                    for pt, j, w in prev:
                        emit_av(pt, j, w)
                    prev = pts
                for pt, j, w in prev:
                    emit_av(pt, j, w)

                # U' -> SBUF [65, 512] bf16 (U rows 0:64, L row 64)
                u_sb = upool.tile([65, 512], BF16, tag="u")
                nc.vector.tensor_copy(u_sb, u_ps)

                row0 = b * S + q0
                nc.sync.dma_start(
                    out=l_d.ap()[row0 : row0 + 512].rearrange("(p c) -> p c", p=1),
                    in_=u_sb[64:65, :],
                )

                # y = U'.T @ Wo_h (unnormalized); K=65 with wo row 64 = 0
                for j2 in range(4):
                    y_ps = ps_misc.tile([128, 512], F32, tag="m")
                    nc.tensor.matmul(
                        y_ps,
                        u_sb[:, j2 * 128 : (j2 + 1) * 128],
                        wo_sb,
                        start=True,
                        stop=True,
                    )
                    y_sb = ypool.tile([128, 512], BF16, tag="y")
                    nc.vector.tensor_copy(y_sb, y_ps)
                    r0 = row0 + j2 * 128
                    nc.sync.dma_start(out=y_d.ap()[r0 : r0 + 128, :], in_=y_sb)

            # Pipeline: proj(tb) immediately enables attn(qb=tb).
            for b in range(B):
                for tb in range(8):
                    proj_block(b, tb)
                    attn_qblock(b, tb)

    nc.compile()
    return nc


def _prep_inputs(x, Wq, bq, Wk, bk, Wv, bv, Wo, bo):
    import ml_dtypes

    bf = ml_dtypes.bfloat16
    xt = np.ascontiguousarray(x.reshape(TOK, D).T).astype(bf)
    mask = np.zeros((128, 4, 512), dtype=np.float32)
    p = np.arange(128)[:, None]
    c = np.arange(512)[None, :]
    for d in range(4):
        mask[:, d, :] = (p + 128 * d <= c).astype(np.float32)
    mask = mask.astype(bf)
    identb = np.eye(64, dtype=np.float32).astype(bf)
    onesb = np.ones((128, NKT), dtype=np.float32).astype(bf)

    in_maps = []
    for h in range(H):
        hs = slice(h * HD, (h + 1) * HD)
        wo_h = np.concatenate(
            [Wo[hs, :], np.zeros((1, D), dtype=np.float32)], axis=0
        )
        in_maps.append(
            {
                "xt": xt,
                "wqk": np.ascontiguousarray(
                    np.concatenate([Wq[:, hs], Wk[:, hs]], axis=1)
                ).astype(bf),
                "wv": np.ascontiguousarray(Wv[:, hs]).astype(bf),
                "wo": wo_h.astype(bf),
                "bqk": np.concatenate([bq[hs], bk[hs]]).reshape(128, 1).astype(
                    np.float32
                ),
                "bv": bv[hs].reshape(HD, 1).astype(np.float32),
                "mask": mask,
                "identb": identb,
                "onesb": onesb,
            }
        )
    return in_maps


def _install_ntff_hook():
    """Register the axon NTFF profiling hook (test-only plumbing)."""
    import types

    try:
        from antenv.axon_hooks import set_axon_ntff_profile_hook  # noqa: F401
    except ImportError:
        m = types.ModuleType("antenv.axon_hooks")
        m._HOOK = None
        m.set_axon_ntff_profile_hook = lambda h: setattr(m, "_HOOK", h)
        m.get_axon_ntff_profile_hook = lambda: m._HOOK
        sys.modules["antenv.axon_hooks"] = m
        import antenv

        antenv.axon_hooks = m
    from antenv.axon_hooks import (
        get_axon_ntff_profile_hook,
        set_axon_ntff_profile_hook,
    )

    if get_axon_ntff_profile_hook() is None:
        import trn_agent_boot.trn_boot as tb

        set_axon_ntff_profile_hook(
            tb._ntff_profile_via_ctypes("/opt/axon/libaxon_pjrt.so")
        )


def kernel(x, Wq, bq, Wk, bk, Wv, bv, Wo, bo, _trace=False):
    x, Wq, bq, Wk, bk, Wv, bv, Wo, bo = (
        np.asarray(a, dtype=np.float32) for a in (x, Wq, bq, Wk, bk, Wv, bv, Wo, bo)
    )
    if "nc" not in _CACHE:
        _CACHE["nc"] = _build()
    nc = _CACHE["nc"]
    in_maps = _prep_inputs(x, Wq, bq, Wk, bk, Wv, bv, Wo, bo)
    kwargs = {}
    if _trace:
        _install_ntff_hook()
        kwargs = dict(trace=True, trace_cores=[0])
    res = run_bass_kernel_spmd(nc, in_maps, core_ids=list(range(8)), **kwargs)
    _CACHE["last_result"] = res
    y = np.zeros((TOK, D), dtype=np.float64)
    for r in res.results:
        y += r["y"].astype(np.float64) / r["l"].astype(np.float64)[:, None]
    y += bo[None, :]
    return y.astype(np.float32).reshape(B, S, D)


# revision 23
# speedup vs baseline: 1.2516x; 1.0030x over previous
"""Causal self-attention (B=2, S=4096, D=512, H=8) on 8 Trainium2 NeuronCores.

Sharding: tensor-parallel over heads. Core h computes head h for both batch
elements: QKV projections for its head, causal flash attention, and its
partial (unnormalized) o_proj contribution y_h = U_h @ Wo[h*64:(h+1)*64, :]
plus the per-query softmax denominators L_h. The host computes
sum_h(y_h / L_h) + bo.

All matmuls run in bf16 (fp32 PSUM accumulation). Score matmuls have K=64
(head dim), so consecutive k-tiles are packed onto the two 64-row groups of
the PE array via tile_position (auto-derived from base partitions) and run
concurrently:
  - qt2 [128, 4096]/batch: Q.T duplicated in partition halves 0:64 / 64:128.
  - ktp [128, 2048]/batch: even k-tiles' K.T in partitions 0:64, odd in
    64:128; column block j*128:(j+1)*128 holds the pair (2j, 2j+1).
  - chunk j: two concurrent matmuls -> st psum [128, 2, 512] (two banks),
    one ACT exp [128, 1024] PSUM->SBUF -> P.T bf16; diagonal chunks get a
    0/1 causal mask multiply on DVE.
  - AV: U'[65, 512] += V'_kt.T @ P.T_kt with V' = [V | ones]; row 64
    accumulates L. AV for chunk j is emitted after the score pair of chunk
    j+1 so each pair stays adjacent in the PE stream.
  - o_proj: y tiles [128q, 512] = U'.T chunk @ Wo_h with K=65 (Wo row 64
    zeroed on host so the L row contributes nothing), bf16 out, DMA'd
    unnormalized together with L (bf16).
"""

import sys

for _p in ("/opt/trn_rl_repo", "/root/.axon_site/_ro/trn_rl_repo"):
    if _p not in sys.path:
        sys.path.insert(0, _p)

import numpy as np

import concourse.bass as bass
import concourse.mybir as mybir
import concourse.tile as tile
from concourse import bacc
from concourse.bass_utils import run_bass_kernel_spmd

B = 2
S = 4096
D = 512
H = 8
HD = 64
TOK = B * S          # 8192
NKT = S // 128       # 32 k-tiles per batch
SCALE = HD ** -0.5

F32 = mybir.dt.float32
BF16 = mybir.dt.bfloat16
F8 = mybir.dt.float8e4

_CACHE = {}


def _build():
    nc = bacc.Bacc("TRN2", target_bir_lowering=False, debug=False, num_devices=8)

    xt_d = nc.dram_tensor("xt", [D, TOK], BF16, kind="ExternalInput")
    wqk_d = nc.dram_tensor("wqk", [D, 128], BF16, kind="ExternalInput")
    wv_d = nc.dram_tensor("wv", [D, HD], BF16, kind="ExternalInput")
    wo_d = nc.dram_tensor("wo", [65, D], BF16, kind="ExternalInput")
    bqk_d = nc.dram_tensor("bqk", [128, 1], F32, kind="ExternalInput")
    bv_d = nc.dram_tensor("bv", [HD, 1], F32, kind="ExternalInput")
    mask_d = nc.dram_tensor("mask", [128, 4, 512], BF16, kind="ExternalInput")
    identb_d = nc.dram_tensor("identb", [64, 64], BF16, kind="ExternalInput")
    onesb_d = nc.dram_tensor("onesb", [128, NKT], BF16, kind="ExternalInput")
    y_d = nc.dram_tensor("y", [TOK, D], BF16, kind="ExternalOutput")
    l_d = nc.dram_tensor("l", [TOK], BF16, kind="ExternalOutput")

    xt_r = xt_d.ap().rearrange("(c p) t -> p c t", p=128)      # [128, 4, 8192]
    wqk_r = wqk_d.ap().rearrange("(c p) m -> p c m", p=128)    # [128, 4, 128]
    wv_r = wv_d.ap().rearrange("(c p) m -> p c m", p=128)      # [128, 4, 64]

    with tile.TileContext(nc) as tc:
        import contextlib

        with contextlib.ExitStack() as ctx:
            singles = ctx.enter_context(tc.tile_pool(name="singles", bufs=1))
            xpool = ctx.enter_context(tc.tile_pool(name="xt", bufs=3))
            ptpool = ctx.enter_context(tc.tile_pool(name="pt", bufs=6))
            upool = ctx.enter_context(tc.tile_pool(name="usb", bufs=2))
            ypool = ctx.enter_context(tc.tile_pool(name="ysb", bufs=4))
            kstpool = ctx.enter_context(tc.tile_pool(name="kst", bufs=2))

            ps_st = ctx.enter_context(
                tc.tile_pool(name="ps_st", bufs=2, space="PSUM")
            )
            ps_u = ctx.enter_context(tc.tile_pool(name="ps_u", bufs=2, space="PSUM"))
            ps_misc = ctx.enter_context(
                tc.tile_pool(name="ps_misc", bufs=2, space="PSUM")
            )

            # --- constants / weights -----------------------------------
            wqk_sb = singles.tile([128, 4, 128], BF16)
            wv_sb = singles.tile([128, 4, HD], BF16)
            wo_sb = singles.tile([65, D], BF16)
            bqk_sb = singles.tile([128, 1], F32)
            bv_sb = singles.tile([HD, 1], F32)
            mask_sb = singles.tile([128, 4, 512], BF16)
            identb = singles.tile([64, 64], BF16)
            nc.sync.dma_start(out=wqk_sb, in_=wqk_r)
            nc.sync.dma_start(out=bqk_sb, in_=bqk_d.ap())
            nc.sync.dma_start(out=wv_sb, in_=wv_r)
            nc.sync.dma_start(out=bv_sb, in_=bv_d.ap())
            nc.sync.dma_start(out=identb, in_=identb_d.ap())
            nc.sync.dma_start(out=mask_sb, in_=mask_d.ap())
            nc.sync.dma_start(out=wo_sb, in_=wo_d.ap())

            # --- persistent per-batch activation buffers ---------------
            qt2 = [
                singles.tile([128, S], BF16, tag=f"qt2_{b}", name=f"qt2_{b}")
                for b in range(B)
            ]
            ktp = [
                singles.tile([128, S // 2], BF16, tag=f"ktp_{b}", name=f"ktp_{b}")
                for b in range(B)
            ]
            vp = [
                singles.tile([128, NKT * 65], BF16, tag=f"vp_{b}", name=f"vp_{b}")
                for b in range(B)
            ]
            for b in range(B):
                nc.sync.dma_start(
                    out=vp[b].rearrange("p (t c) -> p t c", c=65)[:, :, 64:65],
                    in_=onesb_d.ap().rearrange("p (t c) -> p t c", c=1),
                )

            def proj_block(b, tb):
                """Projections for 512 tokens (block tb of batch b)."""
                t0 = b * S + tb * 512
                xt_sb = xpool.tile([128, 4, 512], BF16, tag="xt")
                for c in range(4):
                    nc.sync.dma_start(
                        out=xt_sb[:, c, :], in_=xt_r[:, c, t0 : t0 + 512]
                    )

                qk_ps = ps_misc.tile([128, 512], F32, tag="m")
                for c in range(4):
                    nc.tensor.matmul(
                        qk_ps,
                        wqk_sb[:, c, :],
                        xt_sb[:, c, :],
                        start=(c == 0),
                        stop=(c == 3),
                    )
                vt_ps = ps_misc.tile([HD, 512], F32, tag="m")
                for c in range(4):
                    nc.tensor.matmul(
                        vt_ps,
                        wv_sb[:, c, :],
                        xt_sb[:, c, :],
                        start=(c == 0),
                        stop=(c == 3),
                    )

                # Q.T -> both QT2 halves (+bias) via two DVE adds (the
                # second write is cross-partition-base; avoids the dup DMA
                # latency gating each q-block's first score burst)
                cols = slice(tb * 512, (tb + 1) * 512)
                nc.vector.tensor_scalar_add(
                    qt2[b][0:64, cols], qk_ps[0:64, :], bqk_sb[0:64, 0:1]
                )
                nc.vector.tensor_scalar_add(
                    qt2[b][64:128, cols], qk_ps[0:64, :], bqk_sb[0:64, 0:1]
                )

                # K.T (+bias) scattered straight from PSUM into the
                # even/odd packed ktp layout via two DVE adds (even k-tiles
                # cross-partition-base 64->0; odd stay at 64)
                ksrc = qk_ps[64:128, :].rearrange("p (a b c) -> p a b c", b=2, c=128)
                kdst = ktp[b][:, tb * 256 : (tb + 1) * 256].rearrange(
                    "p (a c) -> p a c", c=128
                )
                nc.vector.tensor_scalar_add(
                    kdst[0:64], ksrc[:, :, 0, :], bqk_sb[64:128, 0:1]
                )
                nc.vector.tensor_scalar_add(
                    kdst[64:128], ksrc[:, :, 1, :], bqk_sb[64:128, 0:1]
                )

                # V.T (+bias, bf16) -> PE transpose to V natural -> V' blocks
                vt_sb = kstpool.tile([HD, 512], BF16, tag="vt")
                nc.vector.tensor_scalar_add(vt_sb, vt_ps, bv_sb[:, 0:1])
                for j in range(4):
                    kt = tb * 4 + j
                    vtr_ps = ps_misc.tile([128, HD], BF16, tag="m")
                    nc.tensor.transpose(
                        vtr_ps, vt_sb[:, j * 128 : (j + 1) * 128], identb
                    )
                    nc.vector.tensor_copy(vp[b][:, kt * 65 : kt * 65 + 64], vtr_ps)

            def attn_qblock(b, qb):
                """Attention + unnormalized o_proj for q-block qb of batch b."""
                q0 = qb * 512
                u_ps = ps_u.tile([65, 512], F32, tag="u")
                n_chunks = 2 * (qb + 1)  # chunks of 2 k-tiles

                def emit_av(pt, j, w):
                    for j2 in range(2):
                        kt = 2 * j + j2
                        nc.tensor.matmul(
                            u_ps[:, 512 - w : 512],
                            vp[b][:, kt * 65 : kt * 65 + 65],
                            pt[:, j2, 0:w],
                            start=(kt == 0),
                            stop=(kt == 2 * n_chunks - 1),
                            skip_group_check=True,
                        )

                # Super-chunks of 2 chunks: the 4 score matmuls (2 packed
                # K=64 pairs) are emitted back-to-back so the PE stays in
                # 64-row tiling mode for the whole burst (mode switches
                # drain the array and defeat pair concurrency); the 128-mode
                # AV matmuls of the previous super-chunk follow the burst.
                # The last (diagonal) chunk covers keys the first 256
                # queries never see: narrow it to the top 256 queries.
                prev = []
                for j2 in range(n_chunks // 2):
                    pts = []
                    sts = []
                    for j in (2 * j2, 2 * j2 + 1):
                        w = 256 if j == n_chunks - 1 else 512
                        qcols = slice(q0 + 512 - w, q0 + 512)
                        st = ps_st.tile([128, 2, 512], F32, tag="st")
                        kcols = slice(j * 128, (j + 1) * 128)
                        nc.tensor.matmul(
                            st[:, 0, 0:w],
                            ktp[b][0:64, kcols],
                            qt2[b][0:64, qcols],
                            start=True,
                            stop=True,
                        )
                        nc.tensor.matmul(
                            st[:, 1, 0:w],
                            ktp[b][64:128, kcols],
                            qt2[b][64:128, qcols],
                            start=True,
                            stop=True,
                        )
                        sts.append((st, w))
                    for j, (st, w) in zip((2 * j2, 2 * j2 + 1), sts):
                        pt = ptpool.tile([128, 2, 512], BF16, tag="pt")
                        nc.scalar.activation(
                            pt[:, :, 0:w],
                            st[:, :, 0:w],
                            mybir.ActivationFunctionType.Exp,
                            scale=SCALE,
                        )
                        if j == n_chunks - 2:  # diagonal, full width
                            nc.vector.tensor_mul(
                                pt, pt, mask_sb[:, 0:2, :]
                            )
                        elif j == n_chunks - 1:  # diagonal, narrowed
                            nc.vector.tensor_mul(
                                pt[:, :, 0:256],
                                pt[:, :, 0:256],
                                mask_sb[:, 0:2, 0:256],
                            )
                        pts.append((pt, j, w))
                    for pt, j, w in prev:
                        emit_av(pt, j, w)
                    prev = pts
                for pt, j, w in prev:
                    emit_av(pt, j, w)

                # U' -> SBUF [65, 512] bf16 (U rows 0:64, L row 64)
                u_sb = upool.tile([65, 512], BF16, tag="u")
                nc.vector.tensor_copy(u_sb, u_ps)

                row0 = b * S + q0
                nc.sync.dma_start(
                    out=l_d.ap()[row0 : row0 + 512].rearrange("(p c) -> p c", p=1),
                    in_=u_sb[64:65, :],
                )

                # y = U'.T @ Wo_h (unnormalized); K=65 with wo row 64 = 0
                for j2 in range(4):
                    y_ps = ps_misc.tile([128, 512], F32, tag="m")
                    nc.tensor.matmul(
                        y_ps,
                        u_sb[:, j2 * 128 : (j2 + 1) * 128],
                        wo_sb,
                        start=True,
                        stop=True,
                    )
                    y_sb = ypool.tile([128, 512], BF16, tag="y")
                    nc.vector.tensor_copy(y_sb, y_ps)
                    r0 = row0 + j2 * 128
                    nc.sync.dma_start(out=y_d.ap()[r0 : r0 + 128, :], in_=y_sb)

            # Pipeline: proj(tb) immediately enables attn(qb=tb).
            for b in range(B):
                for tb in range(8):
                    proj_block(b, tb)
                    attn_qblock(b, tb)

    nc.compile()
    return nc


def _prep_inputs(x, Wq, bq, Wk, bk, Wv, bv, Wo, bo):
    import ml_dtypes

    bf = ml_dtypes.bfloat16
    xt = np.ascontiguousarray(x.reshape(TOK, D).T).astype(bf)
    mask = np.zeros((128, 4, 512), dtype=np.float32)
    p = np.arange(128)[:, None]
    c = np.arange(512)[None, :]
    for d in range(4):
        mask[:, d, :] = (p + 128 * d <= c).astype(np.float32)
    mask = mask.astype(bf)
    identb = np.eye(64, dtype=np.float32).astype(bf)
    onesb = np.ones((128, NKT), dtype=np.float32).astype(bf)

    in_maps = []
    for h in range(H):
        hs = slice(h * HD, (h + 1) * HD)
        wo_h = np.concatenate(
            [Wo[hs, :], np.zeros((1, D), dtype=np.float32)], axis=0
        )
        in_maps.append(
            {
                "xt": xt,
                "wqk": np.ascontiguousarray(
                    np.concatenate([Wq[:, hs], Wk[:, hs]], axis=1)
                ).astype(bf),
                "wv": np.ascontiguousarray(Wv[:, hs]).astype(bf),
                "wo": wo_h.astype(bf),
                "bqk": np.concatenate([bq[hs], bk[hs]]).reshape(128, 1).astype(
                    np.float32
                ),
                "bv": bv[hs].reshape(HD, 1).astype(np.float32),
                "mask": mask,
                "identb": identb,
                "onesb": onesb,
            }
        )
    return in_maps


def _install_ntff_hook():
    """Register the axon NTFF profiling hook (test-only plumbing)."""
    import types

    try:
        from antenv.axon_hooks import set_axon_ntff_profile_hook  # noqa: F401
    except ImportError:
        m = types.ModuleType("antenv.axon_hooks")
        m._HOOK = None
        m.set_axon_ntff_profile_hook = lambda h: setattr(m, "_HOOK", h)
        m.get_axon_ntff_profile_hook = lambda: m._HOOK
        sys.modules["antenv.axon_hooks"] = m
        import antenv

        antenv.axon_hooks = m
    from antenv.axon_hooks import (
        get_axon_ntff_profile_hook,
        set_axon_ntff_profile_hook,
    )

    if get_axon_ntff_profile_hook() is None:
        import trn_agent_boot.trn_boot as tb

        set_axon_ntff_profile_hook(
            tb._ntff_profile_via_ctypes("/opt/axon/libaxon_pjrt.so")
        )


def kernel(x, Wq, bq, Wk, bk, Wv, bv, Wo, bo, _trace=False):
    x, Wq, bq, Wk, bk, Wv, bv, Wo, bo = (
        np.asarray(a, dtype=np.float32) for a in (x, Wq, bq, Wk, bk, Wv, bv, Wo, bo)
    )
    if "nc" not in _CACHE:
        _CACHE["nc"] = _build()
    nc = _CACHE["nc"]
    in_maps = _prep_inputs(x, Wq, bq, Wk, bk, Wv, bv, Wo, bo)
    kwargs = {}
    if _trace:
        _install_ntff_hook()
        kwargs = dict(trace=True, trace_cores=[0])
    res = run_bass_kernel_spmd(nc, in_maps, core_ids=list(range(8)), **kwargs)
    _CACHE["last_result"] = res
    y = np.zeros((TOK, D), dtype=np.float64)
    for r in res.results:
        y += r["y"].astype(np.float64) / r["l"].astype(np.float64)[:, None]
    y += bo[None, :]
    return y.astype(np.float32).reshape(B, S, D)
